# revision 1
# baseline (speedup 1.0000x reference)
"""CTLSTM cell fused kernel for 8 Trainium2 NeuronCores.

Strategy (data-parallel over batch):
  - B=16384 rows sharded 2048/core; weights replicated.
  - Host stages transposed operands so the K contraction dim lands on SBUF
    partitions: xh = [x;ht].T -> [1024, 2048/core], w2 = [Wx;Wh].T ->
    [1024, 3584], both cast to bf16 (PE runs 1 col/cycle and FWL hides the
    weight loads; fp32 would serialize a ~190ns LDWEIGHTS per matmul).
    PSUM accumulation stays fp32.
  - Gate columns are host-permuted to [z, d, i, f, o, i_bar, f_bar] so the
    five sigmoid gates are contiguous: per 128-row subtile ACT runs one
    tanh, one sigmoid(-x) and ONE [128,2560] sigmoid, all in place in a
    contiguous [128,3584] pre-activation mega-tile.
  - bf16 allows N=1024 moving: matmuls compute gate PAIRS into 2-bank
    PSUM tiles; DVE drains each pair with a single fused bias-add.
  - softplus(wd) has no ACT table set; computed as -ln(sigmoid(-wd)).
    sigmoid(-wd) from the main pass is stashed in SBUF; Ln chunks at the
    end are forced (explicit deps) after all main-pass ACT ops so the
    activation table switches exactly once.
"""

import numpy as np
import ml_dtypes

import concourse.bacc as bacc
import concourse.bass as bass
import concourse.mybir as mybir
import concourse.tile as tile
from concourse.tile_rust import add_dep_helper
from concourse.bass_utils import run_bass_kernel_spmd

NCORES = 8
B = 16384
I = 512
H = 512
NG = 7
G = NG * H          # 3584
K2 = I + H          # 1024
P = 128
BS = B // NCORES    # 2048 rows per core
NT = BS // P        # 16 subtiles of 128 rows
SUP = 4             # subtiles per supertile (DMA granularity)
NSUP = NT // SUP

BF16 = mybir.dt.bfloat16
F32 = mybir.dt.float32
AF = mybir.ActivationFunctionType
NPBF16 = ml_dtypes.bfloat16

# gate order in the permuted weight/bias layout (reference order is
# i, f, z, o, d, i_bar, f_bar)
PERM = [2, 4, 0, 1, 3, 5, 6]   # -> z, d, i, f, o, i_bar, f_bar

TRACE = False
LAST_RESULTS = None

_nc_cache = None


def _build():
    nc = bacc.Bacc("TRN2", target_bir_lowering=False, debug=False)

    xh = nc.dram_tensor("xh", [K2, BS], BF16, kind="ExternalInput")
    w2 = nc.dram_tensor("w2", [K2, G], BF16, kind="ExternalInput")
    ct = nc.dram_tensor("ct", [BS, H], F32, kind="ExternalInput")
    bb_d = nc.dram_tensor("bb", [P, G], F32, kind="ExternalInput")

    h_d = nc.dram_tensor("h", [BS, H], F32, kind="ExternalOutput")
    c_d = nc.dram_tensor("c", [BS, H], F32, kind="ExternalOutput")
    cb_d = nc.dram_tensor("cb", [BS, H], F32, kind="ExternalOutput")
    o_d = nc.dram_tensor("o", [BS, H], F32, kind="ExternalOutput")
    dr_d = nc.dram_tensor("dr", [BS, H], F32, kind="ExternalOutput")

    last_sn = None  # final main-pass ACT instruction, gates phase 2

    with tile.TileContext(nc) as tc:
        with (
            tc.tile_pool(name="wp", bufs=1) as wp,
            tc.tile_pool(name="cp", bufs=1) as cp,
            tc.tile_pool(name="sp", bufs=1) as sp,
            tc.tile_pool(name="xp", bufs=2) as xp,
            tc.tile_pool(name="ctp", bufs=4) as ctp,
            tc.tile_pool(name="gp", bufs=2) as gp,
            tc.tile_pool(name="pp", bufs=3, space=bass.MemorySpace.PSUM) as pp,
            tc.tile_pool(name="pps", bufs=2, space=bass.MemorySpace.PSUM) as pps,
        ):
            # resident weights: 8 K-chunks of [128, 3584] bf16
            w_sb = []
            for k in range(8):
                wt = wp.tile([P, G], BF16, tag=f"w{k}")
                nc.sync.dma_start(wt[:], w2[k * P:(k + 1) * P, :])
                w_sb.append(wt)
            # broadcast bias [128, 3584] fp32 (bx+bh, host-staged broadcast)
            bb = cp.tile([P, G], F32, tag="bb")
            nc.sync.dma_start(bb[:], bb_d[:])
            # sigmoid(-wd) stash, one [128, 512] slice per subtile
            stash = sp.tile([P, NT, H], F32, tag="stash")

            for s in range(NSUP):
                xhs = []
                for k in range(8):
                    t_ = xp.tile([P, SUP * P], BF16, tag=f"xh{k}")
                    nc.sync.dma_start(
                        t_[:], xh[k * P:(k + 1) * P, s * SUP * P:(s + 1) * SUP * P]
                    )
                    xhs.append(t_)

                for j in range(SUP):
                    t = s * SUP + j
                    bsl = slice(j * P, (j + 1) * P)
                    rows = slice(t * P, (t + 1) * P)

                    ctj = ctp.tile([P, H], F32, tag="ct")
                    nc.sync.dma_start(ctj[:], ct[rows, :])

                    ga = gp.tile([P, G], F32, tag="ga")

                    # gate pairs (z,d), (i,f), (o,ib) then single (fb); each
                    # pair accumulates in a 2-bank PSUM tile drained by one
                    # fused bias-add
                    for pr in range(3):
                        csl = slice(pr * 2 * H, (pr + 1) * 2 * H)
                        acc = pp.tile([P, 2 * H], F32, tag="accp")
                        for half in range(2):
                            gsl = slice((pr * 2 + half) * H,
                                        (pr * 2 + half + 1) * H)
                            hsl = slice(half * H, (half + 1) * H)
                            for k in range(8):
                                nc.tensor.matmul(
                                    acc[:, hsl], xhs[k][:, bsl], w_sb[k][:, gsl],
                                    start=(k == 0), stop=(k == 7),
                                )
                        nc.vector.tensor_add(ga[:, csl], acc[:], bb[:, csl])
                    csl = slice(6 * H, 7 * H)
                    acc = pps.tile([P, H], F32, tag="accs")
                    for k in range(8):
                        nc.tensor.matmul(
                            acc[:], xhs[k][:, bsl], w_sb[k][:, csl],
                            start=(k == 0), stop=(k == 7),
                        )
                    nc.vector.tensor_add(ga[:, csl], acc[:], bb[:, csl])

                    # permuted gate slices of ga
                    Z = ga[:, 0 * H:1 * H]
                    D = ga[:, 1 * H:2 * H]
                    Ii = ga[:, 2 * H:3 * H]
                    F = ga[:, 3 * H:4 * H]
                    O = ga[:, 4 * H:5 * H]
                    IB = ga[:, 5 * H:6 * H]
                    FB = ga[:, 6 * H:7 * H]

                    nc.scalar.activation(Z, Z, AF.Tanh)
                    nc.scalar.activation(stash[:, t, :], D, AF.Sigmoid,
                                         scale=-1.0)
                    nc.scalar.activation(ga[:, 2 * H:], ga[:, 2 * H:], AF.Sigmoid)

                    nc.sync.dma_start(o_d[rows, :], O)

                    nc.vector.tensor_mul(F, F, ctj[:])    # f*ct
                    nc.vector.tensor_mul(Ii, Ii, Z)       # i*z
                    nc.vector.tensor_add(F, F, Ii)        # c
                    nc.sync.dma_start(c_d[rows, :], F)
                    nc.vector.tensor_mul(IB, IB, Z)       # ib*z
                    last_sn = nc.scalar.activation(Z, F, AF.Tanh)  # tanh(c)
                    nc.vector.tensor_mul(FB, FB, ctj[:])  # fb*ct
                    nc.vector.tensor_add(FB, FB, IB)      # cbar
                    nc.sync.dma_start(cb_d[rows, :], FB)
                    nc.vector.tensor_mul(Z, O, Z)         # h = o*tanh(c)
                    nc.sync.dma_start(h_d[rows, :], Z)

            # phase 2: decay_rate = softplus(wd) = -ln(sigmoid(-wd))
            dr_r = dr_d.rearrange("(n t p) c -> n p t c", t=SUP, p=P)
            for chn in range(NSUP):
                chsl = slice(chn * SUP, (chn + 1) * SUP)
                ln = nc.scalar.activation(stash[:, chsl, :], stash[:, chsl, :],
                                          AF.Ln)
                # keep Ln after every main-pass ACT: one table switch total
                add_dep_helper(ln.ins, last_sn.ins, reason="phase2 after phase1")
                nc.vector.tensor_scalar_mul(stash[:, chsl, :], stash[:, chsl, :],
                                            -1.0)
                nc.sync.dma_start(dr_r[chn], stash[:, chsl, :])

    nc.compile()
    return nc




def kernel(x, ht, ct, Wx, bx, Wh, bh):
    global _nc_cache, LAST_RESULTS
    if _nc_cache is None:
        _nc_cache = _build()
    nc = _nc_cache

    x = np.ascontiguousarray(x, dtype=np.float32)
    ht = np.ascontiguousarray(ht, dtype=np.float32)
    ct = np.ascontiguousarray(ct, dtype=np.float32)

    # host staging: transpose/concat/cast + gate permutation + bias broadcast
    xh_full = np.empty((K2, B), dtype=NPBF16)
    xh_full[:I, :] = x.T.astype(NPBF16)
    xh_full[I:, :] = ht.T.astype(NPBF16)

    WxT = np.asarray(Wx, dtype=np.float32).T   # [512, 3584]
    WhT = np.asarray(Wh, dtype=np.float32).T
    bsum = np.asarray(bx, dtype=np.float32) + np.asarray(bh, dtype=np.float32)
    w2 = np.empty((K2, G), dtype=NPBF16)
    bbp = np.empty(G, dtype=np.float32)
    for n, old in enumerate(PERM):
        dsl = slice(n * H, (n + 1) * H)
        ssl = slice(old * H, (old + 1) * H)
        w2[:I, dsl] = WxT[:, ssl].astype(NPBF16)
        w2[I:, dsl] = WhT[:, ssl].astype(NPBF16)
        bbp[dsl] = bsum[ssl]
    bb = np.ascontiguousarray(np.broadcast_to(bbp[None, :], (P, G)))

    in_maps = []
    for cidx in range(NCORES):
        sl = slice(cidx * BS, (cidx + 1) * BS)
        in_maps.append({
            "xh": np.ascontiguousarray(xh_full[:, sl]),
            "w2": w2,
            "ct": ct[sl],
            "bb": bb,
        })

    res = run_bass_kernel_spmd(nc, in_maps, core_ids=list(range(NCORES)),
                               trace=TRACE)
    LAST_RESULTS = res

    outs = {}
    for name in ("h", "c", "cb", "o", "dr"):
        outs[name] = np.concatenate(
            [res.results[cidx][name] for cidx in range(NCORES)], axis=0
        )
    return outs["h"], outs["c"], outs["cb"], outs["o"], outs["dr"]



# revision 3
# speedup vs baseline: 1.2601x; 1.2601x over previous
"""CTLSTM cell fused kernel for 8 Trainium2 NeuronCores.

Strategy (data-parallel over batch, weight-stationary transposed matmul):
  - B=16384 rows sharded 2048/core; weights replicated.
  - TRANSPOSED layout vs the classic one: gate columns live on PSUM
    partitions, batch on the moving/free dim.  out[g_chunk, b_tile] =
    w2[k, g_chunk].T @ xh[k, b_tile].  The stationary operand is the
    weight block [128,128], reused across 4 consecutive matmuls (2048
    moving batch columns) -> fewer effective weight swaps on the PE.
  - Host stages xh = [x;ht].T as [8, 128, 2048] bf16 k-chunks and
    w2 = [Wx;Wh].T (gate-permuted) as [8, 128, 3584] bf16.
  - Gate rows are host-permuted j-major: for each 128-row h-chunk j,
    the 7 gate chunks [z, d, i, f, o, ib, fb] for that j are adjacent.
    Per (j, batch-tile) unit all elementwise ops line up as [128, 512]
    tiles against ct.T.
  - Bias is per-PARTITION in this layout: every PSUM drain is a single
    ACT op func(psum + bias) straight into SBUF (bias for the d-gate is
    negated: drain computes sigmoid(-(wd)) for the softplus trick).
    No DVE bias-adds, no [128, G] broadcast bias tile.
  - softplus(wd) = -ln(sigmoid(-wd)); sigmoid(-wd) comes out of the
    d-gate drain, Ln runs batched once per j (2 ACT table switches per
    batch, hidden under PE work), then DVE negates and stores dr.
  - DMA issue order: bias, then xh/w interleaved per k-chunk so the PE
    can start accumulating k=0 while k=1.. stream in.
"""

import numpy as np
import ml_dtypes

import concourse.bacc as bacc
import concourse.bass as bass
import concourse.mybir as mybir
import concourse.tile as tile
from concourse.tile_rust import add_dep_helper
from concourse.bass_utils import run_bass_kernel_spmd

NCORES = 8
B = 16384
I = 512
H = 512
NG = 7
G = NG * H          # 3584
K2 = I + H          # 1024
P = 128
BS = B // NCORES    # 2048 rows per core
KC = K2 // P        # 8 k-chunks
NB = BS // 512      # 4 batch tiles of 512
NJ = H // P         # 4 h-chunks
NGC = G // P        # 28 gate chunks

BF16 = mybir.dt.bfloat16
F32 = mybir.dt.float32
AF = mybir.ActivationFunctionType
NPBF16 = ml_dtypes.bfloat16

# gate order within each h-chunk j (reference order: i,f,z,o,d,ib,fb)
# position in our per-j block  ->  original gate index
# [z, d, i, f, o, ib, fb]
PERMJ = [2, 4, 0, 1, 3, 5, 6]
GZ, GD, GI, GF, GO, GIB, GFB = 0, 1, 2, 3, 4, 5, 6

TRACE = False
LAST_RESULTS = None

_nc_cache = None


def _build():
    nc = bacc.Bacc("TRN2", target_bir_lowering=False, debug=False)

    xh_d = nc.dram_tensor("xh", [KC, P, BS], BF16, kind="ExternalInput")
    w_d = nc.dram_tensor("w2", [KC, P, G], BF16, kind="ExternalInput")
    ct_d = nc.dram_tensor("ct", [NJ, P, BS], F32, kind="ExternalInput")
    bias_d = nc.dram_tensor("bias", [P, NGC], F32, kind="ExternalInput")

    h_d = nc.dram_tensor("h", [NJ, P, BS], F32, kind="ExternalOutput")
    c_d = nc.dram_tensor("c", [NJ, P, BS], F32, kind="ExternalOutput")
    cb_d = nc.dram_tensor("cb", [NJ, P, BS], F32, kind="ExternalOutput")
    o_d = nc.dram_tensor("o", [NJ, P, BS], F32, kind="ExternalOutput")
    dr_d = nc.dram_tensor("dr", [NJ, P, BS], F32, kind="ExternalOutput")

    with tile.TileContext(nc) as tc:
        with (
            tc.tile_pool(name="wp", bufs=1) as wp,
            tc.tile_pool(name="bp", bufs=1) as bp,
            tc.tile_pool(name="gp", bufs=4) as gp,
            tc.tile_pool(name="ctp", bufs=8) as ctp,
            tc.tile_pool(name="pp", bufs=8, space=bass.MemorySpace.PSUM) as pp,
        ):
            bias_sb = bp.tile([P, NGC], F32, tag="bias")
            nc.sync.dma_start(bias_sb[:], bias_d[:])

            xh_sb = wp.tile([P, KC, BS], BF16, tag="xh")
            w_sb = wp.tile([P, KC, G], BF16, tag="w")
            for k in range(KC):
                nc.sync.dma_start(xh_sb[:, k, :], xh_d[k])
                nc.sync.dma_start(w_sb[:, k, :], w_d[k])

            for j in range(NJ):
                # ct tiles for this j (needed by DVE only, ~60us of lead)
                cts = []
                for n in range(NB):
                    ctt = ctp.tile([P, 512], F32, tag="ct", name=f"ct{j}_{n}")
                    nc.sync.dma_start(ctt[:], ct_d[j, :, n * 512:(n + 1) * 512])
                    cts.append(ctt)

                gates = [[None] * NB for _ in range(NG)]
                for gi in range(NG):
                    gc = j * NG + gi
                    gsl = slice(gc * P, (gc + 1) * P)
                    accs = [pp.tile([P, 512], F32, tag="acc", name=f"acc{gi}_{n}")
                            for n in range(NB)]
                    for k in range(KC):
                        for n in range(NB):
                            nc.tensor.matmul(
                                accs[n][:],
                                w_sb[:, k, gsl],
                                xh_sb[:, k, n * 512:(n + 1) * 512],
                                start=(k == 0), stop=(k == KC - 1),
                            )
                    func = AF.Tanh if gi == GZ else AF.Sigmoid
                    scale = -1.0 if gi == GD else 1.0
                    for n in range(NB):
                        t = gp.tile([P, 512], F32, tag=f"g{gi}", name=f"g{gi}_{n}")
                        nc.scalar.activation(t[:], accs[n][:], func,
                                             bias=bias_sb[:, gc:gc + 1],
                                             scale=scale)
                        gates[gi][n] = t
                        if gi == GO:
                            nc.sync.dma_start(
                                o_d[j, :, n * 512:(n + 1) * 512], t[:])

                last_tanh = None
                for n in range(NB):
                    nsl = slice(n * 512, (n + 1) * 512)
                    Z = gates[GZ][n]
                    Ii = gates[GI][n]
                    F = gates[GF][n]
                    O = gates[GO][n]
                    IB = gates[GIB][n]
                    FB = gates[GFB][n]
                    ctt = cts[n]

                    nc.vector.tensor_mul(F[:], F[:], ctt[:])    # f*ct
                    nc.vector.tensor_mul(Ii[:], Ii[:], Z[:])    # i*z
                    nc.vector.tensor_add(F[:], F[:], Ii[:])     # c
                    nc.sync.dma_start(c_d[j, :, nsl], F[:])
                    nc.vector.tensor_mul(IB[:], IB[:], Z[:])    # ib*z
                    last_tanh = nc.scalar.activation(Z[:], F[:], AF.Tanh)
                    nc.vector.tensor_mul(FB[:], FB[:], ctt[:])  # fb*ct
                    nc.vector.tensor_add(FB[:], FB[:], IB[:])   # cbar
                    nc.sync.dma_start(cb_d[j, :, nsl], FB[:])
                    nc.vector.tensor_mul(Z[:], O[:], Z[:])      # h
                    nc.sync.dma_start(h_d[j, :, nsl], Z[:])

                # batched softplus tail for this j:
                # dr = -ln(sigmoid(-wd)); keep the Ln ops coherent (after
                # this j's sigmoid/tanh ACT work) so tables switch once
                for n in range(NB):
                    D = gates[GD][n]
                    ln = nc.scalar.activation(D[:], D[:], AF.Ln)
                    add_dep_helper(ln.ins, last_tanh.ins,
                                   reason="ln after main-table acts")
                    nc.vector.tensor_scalar_mul(D[:], D[:], -1.0)
                    nc.sync.dma_start(dr_d[j, :, n * 512:(n + 1) * 512], D[:])

    nc.compile()
    return nc


def kernel(x, ht, ct, Wx, bx, Wh, bh):
    global _nc_cache, LAST_RESULTS
    if _nc_cache is None:
        _nc_cache = _build()
    nc = _nc_cache

    x = np.ascontiguousarray(x, dtype=np.float32)
    ht = np.ascontiguousarray(ht, dtype=np.float32)
    ct = np.ascontiguousarray(ct, dtype=np.float32)

    # xh = [x; ht].T as [k, p, b] chunks, bf16
    xh_full = np.empty((K2, B), dtype=NPBF16)
    xh_full[:I, :] = x.T.astype(NPBF16)
    xh_full[I:, :] = ht.T.astype(NPBF16)
    xh_dev = xh_full.reshape(KC, P, B)

    # w2 = [Wx; Wh].T with gate columns permuted j-major:
    # dst chunk (j*7+gi) <- original gate PERMJ[gi], h-chunk j
    WxT = np.asarray(Wx, dtype=np.float32).T   # [512, 3584]
    WhT = np.asarray(Wh, dtype=np.float32).T
    bsum = np.asarray(bx, dtype=np.float32) + np.asarray(bh, dtype=np.float32)
    w2 = np.empty((K2, G), dtype=NPBF16)
    bias_perm = np.empty(G, dtype=np.float32)
    for j in range(NJ):
        for gi, go in enumerate(PERMJ):
            dsl = slice((j * NG + gi) * P, (j * NG + gi + 1) * P)
            ssl = slice(go * H + j * P, go * H + (j + 1) * P)
            w2[:I, dsl] = WxT[:, ssl].astype(NPBF16)
            w2[I:, dsl] = WhT[:, ssl].astype(NPBF16)
            sgn = -1.0 if gi == GD else 1.0
            bias_perm[dsl] = sgn * bsum[ssl]
    w_dev = w2.reshape(KC, P, G)
    bias_dev = np.ascontiguousarray(bias_perm.reshape(NGC, P).T)

    in_maps = []
    for cidx in range(NCORES):
        sl = slice(cidx * BS, (cidx + 1) * BS)
        in_maps.append({
            "xh": np.ascontiguousarray(xh_dev[:, :, sl]),
            "w2": w_dev,
            "ct": np.ascontiguousarray(ct[sl].T).reshape(NJ, P, BS),
            "bias": bias_dev,
        })

    res = run_bass_kernel_spmd(nc, in_maps, core_ids=list(range(NCORES)),
                               trace=TRACE)
    LAST_RESULTS = res

    outs = {}
    for name in ("h", "c", "cb", "o", "dr"):
        full = np.concatenate(
            [res.results[cidx][name].reshape(H, BS) for cidx in range(NCORES)],
            axis=1,
        )
        outs[name] = np.ascontiguousarray(full.T)
    return outs["h"], outs["c"], outs["cb"], outs["o"], outs["dr"]
